# revision 48
# baseline (speedup 1.0000x reference)
"""Trainium2 Bass kernel for nn_AttnAligner.

Data-parallel over the batch: 8 samples -> 8 NeuronCores, one sample per
core; no collectives.  The (bs, ps, ll) scatter matrix is never
materialized:

    alg.T @ emb_w  ==  attn[b].T @ emb_w[tgt[b]]        (segment_sum is linear)

so the host gathers the 64 embedding rows per sample (zeroing rows where
tgt == 0) and each core runs: tiny f32r matmul -> 4-layer transformer
encoder -> (256, 16000) output projection, all in fp16 with fp32 psum.

Key device-side optimizations (349us -> ~233us on the same HW):
 - fp16 everywhere (same PE rate as bf16, 30x lower error than bf16,
   half the HBM traffic of fp32); output written fp16 and widened on host.
 - Weight layouts give 4-16KB contiguous bytes per partition row; one DMA
   per tensor per layer, double-buffered pools prefetch layer l+1 and the
   projection weights during the transformer: 22 of 32 quarter-chunks
   are resident at projection start (10 in a dedicated pool, 12 riding in
   the w1/w2/wq/wk/wv/wo weight slots that go dead after the last layer),
   and the rest stream with fetches interleaved into the loop so queue
   order never blocks the output writes; projection psums rotate across
   all three psum pools for a 4-deep pipeline.
 - Attention: qT/kT computed d-major directly; q and k share one psum and
   one copy per nb; both jb score-tiles share one psum and ONE exp per
   head (scale 1/sqrt(dh) folded into the exp); softmax denominators come
   free via a ones-column in the v operand; AV+rowsum in one matmul per
   (head, ib, jb); QKV matmuls interleave with s/exp so the scalar engine
   is never the long pole; half of Wo overlaps the second attention pass.
 - Residuals are pre-seeded into psum (matmuls accumulate with
   start=False), removing the add from the LN critical path.
 - LN1 computes only the mean: relu commutes with the positive row scale
   and LN2 cancels it (valid because ln gains/biases are default and
   b1 == b2 == 0; full LN is emitted otherwise).  LN stats use
   bn_stats/bn_aggr; rstd = exp(-0.5*ln(var+eps)) runs on the scalar
   engine whose single ACT table covers Exp/Ln/Relu/Copy (no table swaps).
 - Transposes pack all four [128,128] blocks of a kb-pair into one fp16
   psum tile with a single drain copy; FFN2 runs ib0 skewed inside the
   FFN1 loop and ib1 as one block so LN2(ib0) hides under it.
"""

import numpy as np

BS, LS, LL = 8, 64, 256
PS, D, NL, NH = 16000, 512, 4, 8
DH = D // NH          # 64
DF = 4 * D            # 2048
P = 128
N_IB = LL // P        # 2   seq-partition blocks
N_KB = D // P         # 4   d-partition blocks
N_FB = DF // P        # 16  ffn-partition blocks
PBW = 500             # proj column chunk (<=512 psum free dim)
N_SUB = 4             # sub-chunks per proj group
PGW = PBW * N_SUB     # 2000 columns per proj group
N_PG = PS // PGW      # 8 proj groups
F32 = np.float32
QUAKE = 0x5F3759DF


def _build_bass(use_mask_bias: bool, use_ln_gb: bool, use_b2: bool,
                use_b1: bool):
    import concourse.bass as bass
    import concourse.mybir as mybir
    import concourse.tile as tile
    from contextlib import ExitStack
    from bass_rust import ScopedClock

    f32 = mybir.dt.float32
    f32r = mybir.dt.float32r
    i32 = mybir.dt.int32
    f16 = mybir.dt.float16
    AX = mybir.AxisListType
    OP = mybir.AluOpType
    ACT = mybir.ActivationFunctionType

    class PatchedTC(tile.TileContext):
        """The walrus build in this container rejects >2 sync waits on the
        kernel-tail Drain.  Emit the waits as individual EVSEM wait
        instructions instead, then a waitless drain."""

        def _drain_and_barrier(self, tick_clock, wait_clock):
            dummy = mybir.InstDrain(
                name=f"I-{self.nc.next_id()}", engine=mybir.EngineType.SP
            )
            wait_clock.add_sem_waits(
                dummy, ScopedClock({None: tick_clock.global_clock})
            )
            waits = dummy.sync_info.on_wait if dummy.sync_info is not None else []
            assert self.sems is not None
            handles = {h.name: h for h in self.sems.allocated().values()}
            for w in waits:
                self.nc.sync.wait_ge(handles[w.ant_name], w.wait_value)
            self.nc.sync.drain()
            self.nc.all_engine_barrier()
            popped = self.nc._tile_sem_poison_stack.pop()
            assert popped is self._sem_poison
            self.nc.clear_and_free_semaphores(list(self.sems.allocated().values()))
            self.nc.all_engine_barrier()

    nc = bass.Bass("TRN2", target_bir_lowering=False, debug=False)

    def din(name, shape, dt=None):
        return nc.dram_tensor(name, list(shape), dt or f32,
                              kind="ExternalInput").ap()

    attn_d = din("attn", (LS, LL), f32r)
    embg_d = din("embg", (LS, D), f32r)
    pe_d = din("pe", (N_IB, P, D))
    identh_d = din("identh", (P, P), f16)
    wq_d = din("wq", (NL, P, N_KB, D), f16)
    wk_d = din("wk", (NL, P, N_KB, D), f16)
    wv_d = din("wv", (NL, P, N_KB, D), f16)
    wo_d = din("wo", (NL, P, N_KB, D), f16)
    w1_d = din("w1", (NL, P, N_KB, N_FB, P), f16)
    w2_d = din("w2", (NL, P, N_FB, D), f16)
    if use_b1:
        b1_d = din("b1", (NL, P, N_FB))
    if use_ln_gb:
        lnw_d = din("lnw", (NL, 4, P, D))
    if use_b2:
        b2r_d = din("b2r", (NL, P, D))
    projr_d = din("projr", (N_PG * 4, P, N_KB, PBW), f16)
    if use_mask_bias:
        maskbT_d = din("maskbT", (N_IB, P, 1))
    out_d = nc.dram_tensor("out", [N_IB, N_PG, P, PGW], f16,
                           kind="ExternalOutput").ap()

    with PatchedTC(nc) as tc, ExitStack() as stk:
        const = stk.enter_context(tc.tile_pool(name="const", bufs=1))
        sb = stk.enter_context(tc.tile_pool(name="sb", bufs=2))
        pp_med = stk.enter_context(tc.tile_pool(name="pp_med", bufs=4, space="PSUM"))
        pp_av = stk.enter_context(tc.tile_pool(name="pp_av", bufs=2, space="PSUM"))
        pp_big = stk.enter_context(tc.tile_pool(name="pp_big", bufs=2, space="PSUM"))

        # ---- constants ----
        attn_sb = const.tile([LS, LL], f32r, tag="attn", name="attn")
        nc.scalar.dma_start(attn_sb[:], attn_d[:])
        embg_sb = const.tile([LS, D], f32r, tag="embg", name="embg")
        nc.scalar.dma_start(embg_sb[:], embg_d[:])
        identh = const.tile([P, P], f16, tag="identh", name="identh")
        nc.sync.dma_start(identh[:], identh_d[:])
        pe_sb = [const.tile([P, D], f32, tag=f"pe{ib}", name=f"pe{ib}")
                 for ib in range(N_IB)]
        for ib in range(N_IB):
            nc.sync.dma_start(pe_sb[ib][:], pe_d[ib])
        w0_early = {}
        for nm, dsrc in (("wq", wq_d), ("wk", wk_d)):
            t = sb.tile([P, N_KB, D], f16, tag=nm, name=nm, bufs=2)
            nc.sync.dma_start(t[:], dsrc[0])
            w0_early[nm] = t
        if use_mask_bias:
            maskbT_sb = [const.tile([P, 1], f32, tag=f"maskbT{j}", name=f"maskbT{j}")
                         for j in range(N_IB)]
            for j in range(N_IB):
                nc.sync.dma_start(maskbT_sb[j][:], maskbT_d[j])
        epsb = const.tile([P, 1], f32, tag="epsb", name="epsb")
        nc.vector.memset(epsb[:], 1e-5)
        # v with a persistent ones column at [:, :, DH] per jb
        v_pad = [const.tile([P, NH, DH + 1], f16, tag=f"vp{jb}", name=f"vp{jb}")
                 for jb in range(N_IB)]
        for jb in range(N_IB):
            nc.vector.memset(v_pad[jb][:], 0.0)
            nc.vector.memset(v_pad[jb][:, :, DH:DH + 1], 1.0)

        def transpose_f16(src_tiles, dst_tag, bufs=2, dst=None, pairs=None):
            """src: N_IB fp16 [128, 512] row-major tiles -> 2 fp16
            [128, 2, 256] d-major pair-tiles (kb = 2*pair + sub).  All four
            [128,128] transposed blocks of a kb-pair land in one [128, 512]
            fp16 psum tile, drained by a single copy."""
            if dst is None:
                dst = [sb.tile([P, 2, LL], f16, tag=f"{dst_tag}{p}",
                               name=f"{dst_tag}{p}", bufs=bufs)
                       for p in range(N_KB // 2)]
            if pairs is None:
                pairs = range(N_KB // 2)
            pt = {}
            for p in pairs:
                t = pp_med.tile([P, 4, P], f16, tag="med", name="med")
                for sub in range(2):
                    kb = 2 * p + sub
                    nc.tensor.transpose(t[:, 2 * sub, :],
                                        src_tiles[0][:, kb * P:(kb + 1) * P],
                                        identh[:])
                pt[p] = t
            for i, p in enumerate(pairs):
                for sub in range(2):
                    kb = 2 * p + sub
                    nc.tensor.transpose(pt[p][:, 2 * sub + 1, :],
                                        src_tiles[1][:, kb * P:(kb + 1) * P],
                                        identh[:])
                if i % 2 == 0:
                    nc.vector.tensor_copy(dst[p][:], pt[p][:])
                else:
                    nc.scalar.activation(dst[p][:], pt[p][:], ACT.Copy)
            return dst

        def t_ap(dst, kb):
            # [128, 256] d-major view of column-block kb
            p, sub = kb // 2, kb % 2
            return dst[p][:, sub, :]

        def layer_norm(ps_in, gw, bw, out_tag, bufs, skip_scale=False):
            """y = LN(ps_in) (ps_in already holds residual + f).  With
            skip_scale, only the mean is removed: downstream relu/matmuls
            commute with the positive row scale and the next LN cancels it
            (valid when ln gains/biases are default and b1 == b2 == 0)."""
            st6 = sb.tile([P, 6], f32, tag="st6", name="st6", bufs=4)
            nc.vector.bn_stats(st6[:], ps_in)
            mv = sb.tile([P, 2], f32, tag="mv", name="mv", bufs=4)
            nc.vector.bn_aggr(mv[:], st6[:])
            nmu = sb.tile([P, 1], f32, tag="nmu", name="nmu", bufs=4)
            nc.vector.tensor_scalar(nmu[:], mv[:, 0:1], -1.0, None, OP.mult)
            out = sb.tile([P, D], f16, tag=out_tag, name=out_tag, bufs=bufs)
            rstd = sb.tile([P, 1], f32, tag="rstd", name="rstd", bufs=4)
            if skip_scale:
                nc.vector.memset(rstd[:], 1.0)
            else:
                lnv = sb.tile([P, 1], f32, tag="lnv", name="lnv", bufs=4)
                nc.scalar.activation(lnv[:], mv[:, 1:2], ACT.Ln, bias=epsb[:])
                nc.scalar.activation(rstd[:], lnv[:], ACT.Exp, scale=-0.5)
            if skip_scale:
                nmr = nmu
            else:
                nmr = sb.tile([P, 1], f32, tag="nmr", name="nmr", bufs=4)
                nc.scalar.activation(nmr[:], nmu[:], ACT.Identity,
                                     scale=rstd[:])
            if use_ln_gb:
                yf = sb.tile([P, D], f32, tag=f"{out_tag}_n", name=f"{out_tag}_n",
                             bufs=2)
                nc.scalar.activation(yf[:], ps_in, ACT.Identity, bias=nmr[:],
                                     scale=rstd[:])
                nc.vector.tensor_tensor(yf[:], yf[:], gw[:], OP.mult)
                nc.vector.tensor_tensor(out[:], yf[:], bw[:], OP.add)
            else:
                nc.scalar.activation(out[:, 0:D // 2], ps_in[:, 0:D // 2],
                                     ACT.Identity, bias=nmr[:], scale=rstd[:])
                nc.scalar.activation(out[:, D // 2:D], ps_in[:, D // 2:D],
                                     ACT.Identity, bias=nmr[:], scale=rstd[:])
            return out

        ln1_fast = not (use_ln_gb or use_b1 or use_b2)

        # ---- stage 0: x = attn.T @ embg + pe  (fp16 out) ----
        x = [sb.tile([P, D], f16, tag=f"x{ib}", name=f"x{ib}", bufs=2)
             for ib in range(N_IB)]
        for ib in range(N_IB):
            ps_x = pp_big.tile([P, 512], f32, tag="big", name="big")
            nc.tensor.matmul(ps_x[:, :D], attn_sb[:, ib * P:(ib + 1) * P],
                             embg_sb[:], start=True, stop=True)
            nc.vector.tensor_tensor(x[ib][:], ps_x[:, :D], pe_sb[ib][:], OP.add)

        # ---- proj weights: 32 quarter-chunks of [P, N_KB, 500] (4KB rows).
        # 10 live in a dedicated pool, 12 ride in the w1/w2/wq/wk/wv/wo
        # weight slots once those go dead; only 10 stream during the phase.
        N_PC = N_PG * 4
        projt = [None] * N_PC
        proj_pool = stk.enter_context(tc.tile_pool(name="projp", bufs=10))

        def fetch_proj(qc):
            projt[qc] = proj_pool.tile([P, N_KB, PBW], f16, tag="projt",
                                       name=f"projt{qc}")
            eng = nc.scalar if qc % 2 == 0 else nc.sync
            eng.dma_start(projt[qc][:], projr_d[qc])

        # ---- transformer layers ----
        for l in range(NL):
            if l == 0:
                wq_t = w0_early["wq"]
                wk_t = w0_early["wk"]
            else:
                wq_t = sb.tile([P, N_KB, D], f16, tag="wq", name="wq", bufs=2)
                nc.sync.dma_start(wq_t[:], wq_d[l])
                wk_t = sb.tile([P, N_KB, D], f16, tag="wk", name="wk", bufs=2)
                nc.sync.dma_start(wk_t[:], wk_d[l])
            wv_t = sb.tile([P, N_KB, D], f16, tag="wv", name="wv", bufs=2)
            nc.sync.dma_start(wv_t[:], wv_d[l])
            wo_t = sb.tile([P, N_KB, D], f16, tag="wo", name="wo", bufs=2)
            nc.sync.dma_start(wo_t[:], wo_d[l])
            w1_t = sb.tile([P, N_KB, N_FB, P], f16, tag="w1", name="w1", bufs=2)
            nc.sync.dma_start(w1_t[:], w1_d[l])
            w2_t = sb.tile([P, N_FB, D], f16, tag="w2", name="w2", bufs=2)
            nc.sync.dma_start(w2_t[:], w2_d[l])
            if use_b1:
                b1_t = sb.tile([P, N_FB], f32, tag="b1", name="b1", bufs=2)
                nc.sync.dma_start(b1_t[:], b1_d[l])
            if use_ln_gb:
                ln_t = [sb.tile([P, D], f32, tag=f"lnw{j}", name=f"lnw{j}", bufs=2)
                        for j in range(4)]
                for j in range(4):
                    nc.sync.dma_start(ln_t[j][:], lnw_d[l, j])
            else:
                ln_t = [None] * 4
            if use_b2:
                b2_t = sb.tile([P, D], f32, tag="b2r", name="b2r", bufs=2)
                nc.sync.dma_start(b2_t[:], b2r_d[l])
            if l < 2:
                for c in range(4 * l, 4 * l + 4):
                    fetch_proj(c)
            elif l == 2:
                fetch_proj(8)
                fetch_proj(9)

            xT = transpose_f16(x, "xT")

            # QKV interleaved with s/exp so the scalar engine's 8 exps
            # start early; q and k share one [128, 512] psum per nb and one
            # copy into qkT [128, 2, 256]; s(h) pairs both jb into one psum
            # with a single exp into uT [128, 2, 256].
            qkT = [None] * N_KB
            uT = {}
            o_sb = [sb.tile([P, D], f16, tag=f"o{ib}", name=f"o{ib}", bufs=2)
                    for ib in range(N_IB)]
            rinv = [sb.tile([P, NH], f32, tag=f"rinv{ib}", name=f"rinv{ib}",
                            bufs=2) for ib in range(N_IB)]

            def emit_qkT(nb):
                ps_qk = pp_med.tile([P, 2, LL], f32, tag="med", name="med")
                for kb in range(N_KB):
                    nc.tensor.matmul(ps_qk[:, 0, :],
                                     wq_t[:, kb, nb * P:(nb + 1) * P],
                                     t_ap(xT, kb),
                                     start=(kb == 0), stop=(kb == N_KB - 1))
                for kb in range(N_KB):
                    nc.tensor.matmul(ps_qk[:, 1, :],
                                     wk_t[:, kb, nb * P:(nb + 1) * P],
                                     t_ap(xT, kb),
                                     start=(kb == 0), stop=(kb == N_KB - 1))
                t = sb.tile([P, 2, LL], f16, tag=f"qkT{nb}", name=f"qkT{nb}",
                            bufs=2)
                nc.vector.tensor_copy(t[:], ps_qk[:])
                qkT[nb] = t

            def emit_v(jb):
                ps_v = pp_big.tile([P, 512], f32, tag="big", name="big")
                for kb in range(N_KB):
                    nc.tensor.matmul(ps_v[:, :D],
                                     t_ap(xT, kb)[:, jb * P:(jb + 1) * P],
                                     wv_t[:, kb, :],
                                     start=(kb == 0), stop=(kb == N_KB - 1))
                nc.vector.tensor_copy(
                    v_pad[jb][:, :, 0:DH],
                    ps_v[:, :D].rearrange("p (h e) -> p h e", h=NH))

            def emit_s_exp(h):
                nb, half = h // 2, h % 2
                rs = slice(half * DH, (half + 1) * DH)
                ps_s = pp_med.tile([P, 2, LL], f32, tag="med", name="med")
                for jb in range(N_IB):
                    nc.tensor.matmul(ps_s[:, jb, :],
                                     qkT[nb][rs, 1, jb * P:(jb + 1) * P],
                                     qkT[nb][rs, 0, :],
                                     start=True, stop=True)
                    if use_mask_bias:
                        nc.vector.tensor_scalar_add(ps_s[:, jb, :],
                                                    ps_s[:, jb, :],
                                                    maskbT_sb[jb][:])
                ut = sb.tile([P, 2, LL], f16, tag="uT", name="uT", bufs=6)
                nc.scalar.activation(ut[:], ps_s[:], ACT.Exp,
                                     scale=1.0 / float(np.sqrt(DH)))
                uT[h] = ut

            def emit_av(heads):
                ps_av = [pp_av.tile([P, NH // 2, DH + 1], f32, tag="av",
                                    name="av") for _ in range(N_IB)]
                for h in heads:
                    hh = h % 4
                    for ib in range(N_IB):
                        for jb in range(N_IB):
                            nc.tensor.matmul(ps_av[ib][:, hh, :],
                                             uT[h][:, jb, ib * P:(ib + 1) * P],
                                             v_pad[jb][:, h, :],
                                             start=(jb == 0),
                                             stop=(jb == N_IB - 1))
                hq = heads[0] // 4
                for ib in range(N_IB):
                    nc.vector.reciprocal(rinv[ib][:, hq * 4:(hq + 1) * 4],
                                         ps_av[ib][:, :, DH:DH + 1])
                    for h in heads:
                        hh = h % 4
                        nc.vector.tensor_scalar(
                            o_sb[ib][:, h * DH:(h + 1) * DH],
                            ps_av[ib][:, hh, 0:DH],
                            rinv[ib][:, h:h + 1], None, OP.mult)

            emit_qkT(0)
            emit_qkT(1)
            emit_s_exp(0)
            emit_s_exp(1)
            emit_qkT(2)
            emit_s_exp(2)
            emit_s_exp(3)
            emit_qkT(3)
            emit_v(0)
            emit_v(1)
            ps_ao = []
            for ib in range(N_IB):
                t = pp_big.tile([P, 512], f32, tag="big", name="big")
                nc.vector.tensor_copy(t[:, :D], x[ib][:])
                ps_ao.append(t)
            emit_av([0, 1, 2, 3])
            for h in range(4, NH):
                emit_s_exp(h)
            # heads 0-3 -> oT pair 0 -> partial Wo while heads 4-7 finish
            oT = transpose_f16(o_sb, "oT", pairs=[0])
            for ib in range(N_IB):
                for kb in (0, 1):
                    nc.tensor.matmul(ps_ao[ib][:, :D],
                                     t_ap(oT, kb)[:, ib * P:(ib + 1) * P],
                                     wo_t[:, kb, :],
                                     start=False, stop=False,
                                     skip_group_check=True)
            emit_av([4, 5, 6, 7])
            oT = transpose_f16(o_sb, "oT", dst=oT, pairs=[1])

            # finish Wo (kb 2,3) and LN1 (mean-only when ln1_fast)
            x1 = []
            for ib in range(N_IB):
                for kb in (2, 3):
                    nc.tensor.matmul(ps_ao[ib][:, :D],
                                     t_ap(oT, kb)[:, ib * P:(ib + 1) * P],
                                     wo_t[:, kb, :],
                                     start=False, stop=(kb == 3),
                                     skip_group_check=True)
                x1.append(layer_norm(ps_ao[ib][:, :D], ln_t[0], ln_t[1],
                                     f"x1_{ib}", 2, skip_scale=ln1_fast))
            x1T = transpose_f16(x1, "x1T")

            # FFN with one-stage skew: FFN1(fb) ... FFN2(ib0, fb-1); x1
            # pre-seeded into the FFN2 psums; ib1's FFN2 runs as one block
            # so LN2(ib0) hides under it.
            ps_f = []
            for ib in range(N_IB):
                t = pp_big.tile([P, 512], f32, tag="big", name="big")
                nc.vector.tensor_copy(t[:, :D], x1[ib][:])
                ps_f.append(t)
            hT = [None] * N_FB

            def emit_ffn2(ib, fb):
                nc.tensor.matmul(ps_f[ib][:, :D],
                                 hT[fb][:, ib * P:(ib + 1) * P],
                                 w2_t[:, fb, :],
                                 start=False, stop=(fb == N_FB - 1),
                                 skip_group_check=True)

            for fb in range(N_FB):
                ps_h = pp_med.tile([P, LL], f32, tag="med", name="med")
                for kb in range(N_KB):
                    nc.tensor.matmul(ps_h[:], w1_t[:, kb, fb, :],
                                     t_ap(x1T, kb),
                                     start=(kb == 0), stop=(kb == N_KB - 1))
                ht = sb.tile([P, LL], f16, tag="hT", name="hT", bufs=N_FB)
                if use_b1:
                    nc.vector.tensor_scalar(ht[:], ps_h[:], b1_t[:, fb:fb + 1],
                                            0.0, OP.add, OP.max)
                elif fb % 2 == 0:
                    nc.scalar.activation(ht[:], ps_h[:], ACT.Relu)
                else:
                    nc.vector.tensor_scalar(ht[:], ps_h[:], 0.0, None, OP.max)
                hT[fb] = ht
                if fb >= 1:
                    emit_ffn2(0, fb - 1)
            emit_ffn2(0, N_FB - 1)

            # LN2(ib0) runs on DVE/ACT while the PE does all of ib1's FFN2
            x_next = [None, None]
            if use_b2:
                nc.vector.tensor_tensor(ps_f[0][:, :D], ps_f[0][:, :D],
                                        b2_t[:], OP.add)
            x_next[0] = layer_norm(ps_f[0][:, :D], ln_t[2], ln_t[3], "x0", 2)
            for fb in range(N_FB):
                emit_ffn2(1, fb)
            if use_b2:
                nc.vector.tensor_tensor(ps_f[1][:, :D], ps_f[1][:, :D],
                                        b2_t[:], OP.add)
            x_next[1] = layer_norm(ps_f[1][:, :D], ln_t[2], ln_t[3], "x1", 2)
            x = x_next

        # ---- final projection: out = x @ projT (streamed, fp16 out) ----
        slot_q = {}
        for i, (q0, nq, wtag) in enumerate(
                [(16, 4, "w1"), (20, 4, "w2"), (24, 1, "wq"), (25, 1, "wk"),
                 (26, 1, "wv"), (27, 1, "wo")]):
            t = sb.tile([P, nq, N_KB, PBW], f16, tag=wtag, name=f"pq{q0}")
            for j in range(nq):
                eng = nc.scalar if (i + j) % 2 == 0 else nc.sync
                eng.dma_start(t[:, j], projr_d[q0 + j])
                slot_q[q0 + j] = t[:, j]
        STREAM = [10, 11, 12, 13, 14, 15, 28, 29, 30, 31]
        xTf = transpose_f16(x, "xT")
        n = 0
        for g in range(N_PG):
            if g >= 1:
                for qc in STREAM[2 * (g - 1):2 * g]:
                    fetch_proj(qc)
            for ib in range(N_IB):
                stage = sb.tile([P, PGW], f16, tag="ostage", name="ostage", bufs=3)
                for sub in range(N_SUB):
                    qc = 4 * g + sub
                    mv = slot_q[qc] if qc in slot_q else projt[qc][:]
                    if sub == 0:
                        ps_p = pp_big.tile([P, 512], f32, tag="big", name="big")
                    elif sub == 2:
                        ps_p = pp_av.tile([P, 500], f32, tag="av", name="av")
                    else:
                        ps_p = pp_med.tile([P, 500], f32, tag="med", name="med")
                    for kb in range(N_KB):
                        nc.tensor.matmul(
                            ps_p[:, :PBW],
                            t_ap(xTf, kb)[:, ib * P:(ib + 1) * P],
                            mv[:, kb, :],
                            start=(kb == 0), stop=(kb == N_KB - 1))
                    if n % 2 == 0:
                        nc.scalar.activation(stage[:, sub * PBW:(sub + 1) * PBW],
                                             ps_p[:, :PBW], ACT.Copy)
                    else:
                        nc.vector.tensor_copy(stage[:, sub * PBW:(sub + 1) * PBW],
                                              ps_p[:, :PBW])
                    n += 1
                if g == N_PG - 1 and ib == N_IB - 1:
                    nc.sync.dma_start(out_d[ib, g][:, 0:PGW // 2],
                                      stage[:, 0:PGW // 2])
                    nc.scalar.dma_start(out_d[ib, g][:, PGW // 2:PGW],
                                        stage[:, PGW // 2:PGW])
                else:
                    eng = nc.sync if (g + ib) % 2 == 0 else nc.scalar
                    eng.dma_start(out_d[ib, g], stage[:])

    _split_excess_waits(nc, mybir, maxw=1)
    return nc


def _split_excess_waits(nc, mybir, maxw=1):
    """This container's walrus rejects instructions with more than `maxw`
    sync waits.  Move excess immediate sem waits onto standalone EVSEM
    instructions inserted just before, on the same engine."""
    for fn in nc.m.functions:
        for blk in fn.blocks:
            out = []
            changed = False
            for ins in blk.instructions:
                si = ins.sync_info
                if si is not None and len(si.on_wait) > maxw:
                    waits = list(si.on_wait)
                    movable = [w for w in waits
                               if w.sync_type == "semaphore" and w.wait_reg is None]
                    fixed = [w for w in waits if w not in movable]
                    keep_budget = maxw - len(fixed)
                    assert keep_budget >= 0, f"unmovable waits exceed limit: {ins}"
                    keep = movable[len(movable) - keep_budget:] if keep_budget else []
                    move = movable[:len(movable) - keep_budget]
                    for i in range(0, len(move), maxw):
                        ev = mybir.InstEventSemaphore(
                            name=f"I-{nc.next_id()}", engine=ins.engine)
                        ev.sync_info = mybir.SyncInfo(
                            on_wait=move[i:i + maxw], on_update=[])
                        nc.register_instruction(ev, overwrite=True)
                        out.append(ev)
                    ins.sync_info = mybir.SyncInfo(
                        on_wait=fixed + keep, on_update=list(si.on_update))
                    changed = True
                out.append(ins)
            if changed:
                blk.instructions = out


def _host_prepare(inputs):
    """Returns (shared_map, per_core_list, flags)."""
    g = {k: np.asarray(v) for k, v in inputs.items()}
    attn, mask, tgt = g["attn"], g["mask"], g["tgt"]
    emb_w, proj_w = np.asarray(g["emb_w"], F32), np.asarray(g["proj_w"], F32)
    f16 = np.float16

    # positional encoding (matches reference)
    pos = np.arange(LL, dtype=F32)[:, None]
    div = np.exp(np.arange(0, D, 2, dtype=F32) * (-np.log(10000.0) / D))
    pe = np.zeros((LL, D), F32)
    pe[:, 0::2] = np.sin(pos * div)
    pe[:, 1::2] = np.cos(pos * div)
    pe_r = np.ascontiguousarray(pe.reshape(N_IB, P, D))

    def kmajor(w):  # (512, N) -> (128, 4, N)
        n = w.shape[1]
        return np.ascontiguousarray(
            np.asarray(w, F32).reshape(N_KB, P, n).transpose(1, 0, 2))

    wq = np.stack([kmajor(g["Wq"][l]) for l in range(NL)])
    wk = np.stack([kmajor(g["Wk"][l]) for l in range(NL)])
    wv = np.stack([kmajor(g["Wv"][l]) for l in range(NL)])
    wo = np.stack([kmajor(g["Wo"][l]) for l in range(NL)])
    # W1: (512, 2048) -> (128, 4, 16, 128)
    w1 = np.stack([
        np.ascontiguousarray(
            np.asarray(g["W1"][l], F32).reshape(N_KB, P, N_FB, P)
            .transpose(1, 0, 2, 3))
        for l in range(NL)])
    # W2: (2048, 512) -> (128, 16, 512)
    w2 = np.stack([
        np.ascontiguousarray(
            np.asarray(g["W2"][l], F32).reshape(N_FB, P, D).transpose(1, 0, 2))
        for l in range(NL)])
    # projT: (512, 16000) -> (32, 128, 4, 500)
    projr = np.ascontiguousarray(
        proj_w.T.reshape(N_KB, P, N_PG * 4, PBW).transpose(2, 1, 0, 3))

    ln_vecs = [np.asarray(g[k], F32) for k in ("ln1_g", "ln1_b", "ln2_g", "ln2_b")]
    use_ln_gb = not (np.all(ln_vecs[0] == 1) and np.all(ln_vecs[1] == 0)
                     and np.all(ln_vecs[2] == 1) and np.all(ln_vecs[3] == 0))
    use_b2 = bool(np.any(np.asarray(g["b2"], F32) != 0))
    use_b1 = bool(np.any(np.asarray(g["b1"], F32) != 0))
    use_mask_bias = not bool(np.asarray(mask).all())

    ones = np.ones((P, 1), F32)
    shared = dict(pe=pe_r, identh=np.eye(P, dtype=f16),
                  wq=wq.astype(f16), wk=wk.astype(f16), wv=wv.astype(f16),
                  wo=wo.astype(f16), w1=w1.astype(f16), w2=w2.astype(f16),
                  projr=projr.astype(f16))
    if use_b1:
        shared["b1"] = np.stack([
            np.ascontiguousarray(np.asarray(g["b1"][l], F32).reshape(N_FB, P).T)
            for l in range(NL)])
    if use_ln_gb:
        shared["lnw"] = np.stack([
            np.stack([ones * v[l][None, :] for v in ln_vecs])
            for l in range(NL)])
    if use_b2:
        shared["b2r"] = np.stack([ones * np.asarray(g["b2"][l], F32)[None, :]
                                  for l in range(NL)])

    per_core = []
    sqrt_d = np.sqrt(np.float32(D))
    for b in range(BS):
        tg = np.asarray(tgt[b]).astype(np.int64)
        embg = emb_w[tg] * (tg != 0)[:, None].astype(F32) * sqrt_d
        m = dict(attn=np.ascontiguousarray(np.asarray(attn[b], F32)),
                 embg=np.ascontiguousarray(embg.astype(F32)))
        if use_mask_bias:
            mb = np.where(np.asarray(mask[b]), 0.0, -1e9).astype(F32)
            m["maskbT"] = np.ascontiguousarray(mb.reshape(N_IB, P, 1))
        per_core.append(m)
    return shared, per_core, (use_mask_bias, use_ln_gb, use_b2, use_b1)


def _unshard(results):
    """results: list of per-core {'out': [N_IB, N_PG, P, PGW] fp16}."""
    outs = []
    for b in range(BS):
        o = np.asarray(results[b]["out"])
        o = o.transpose(0, 2, 1, 3).reshape(LL, PS)
        outs.append(o.astype(F32))
    return np.stack(outs)


def kernel(**inputs):
    # If the environment sets BASS_TRACE but the container lacks the axon
    # NTFF hook module, degrade gracefully instead of crashing on import.
    import sys, types
    try:
        import antenv.axon_hooks  # noqa: F401
    except ImportError:
        m = types.ModuleType("antenv.axon_hooks")
        m.get_axon_ntff_profile_hook = lambda: None
        m.set_axon_ntff_profile_hook = lambda h: None
        sys.modules["antenv.axon_hooks"] = m
        try:
            import antenv
            antenv.axon_hooks = m
        except ImportError:
            pass

    from concourse import bass_utils

    shared, per_core, flags = _host_prepare(inputs)
    nc = _build_bass(*flags)
    in_maps = [{**shared, **pc} for pc in per_core]
    res = bass_utils.run_bass_kernel_spmd(nc, in_maps, core_ids=list(range(BS)),
                                          trace=False)
    return _unshard(res.results)
